# revision 11
# baseline (speedup 1.0000x reference)
"""Multi-head attention forward for Trainium2, 8 NeuronCores — v2.

Problem: B=4, S=2048, D=1024, H=16 heads (dk=64), fp32 reference:
  q/k/v = x @ W{q,k,v}^T + b ; heads split; softmax(q k^T / 8) v ; out @ Wo^T + bo

Sharding: 8 cores = 4 batches x 2 head-groups (8 heads each); host sums the
two output-projection partials per batch.

All-fp16 matmuls (fp32 PSUM accumulate).  Math folds:
  - k bias dropped (constant-per-query shift cancels in softmax)
  - v bias folded into bo on host (bo_eff = bo + bv @ Wo_slice^T)
  - softmax denominator via 4 ones-columns appended to V (M=68 PV matmuls);
    denominator rows land in PSUM rows 64:67, normalized with
    reciprocal + stream_shuffle broadcast + tensor_tensor multiply on DVE.
Schedule: per-head phase B (scores -> exp on ACT -> PV) software-pipelined
with SDEPTH=2 score/exp lead; next pair's Q/K projection matmuls drip in as
PE fillers (one per kti step) so the PE never idles while ACT runs exp.
"""

import sys

sys.path.insert(0, "/opt/trn_rl_repo")

import numpy as np

import concourse.bass as bass  # noqa: F401
import concourse.mybir as mybir
import concourse.tile as tile
from concourse import bacc, bass_utils  # noqa: F401

B, S, D, H = 4, 2048, 1024, 16
DK = D // H          # 64
G = 2                # head groups (tensor-parallel factor)
DL = D // G          # 512 local features per core
NH = H // G          # 8 heads per core
NPAIR = NH // 2      # 4 head-pairs
EC = D // 128        # 8 contraction chunks for projections
ST = S // 128        # 16 s-tiles / key tiles
VW = 68              # V block width per head: 64 features + 4 ones columns

F32 = mybir.dt.float32
F16 = mybir.dt.float16

# Schraudolph constants (odd heads' exp on DVE); scores range +-6.9 so the
# int16 intermediate s/8 * 2^10/ln2 + B16 stays within [5.1e3, 2.6e4].
SCHR_A8 = 1024.0 / 0.6931471805599453 / 8.0   # 2^10/ln2 / 8
SCHR_B16 = 15.0 * 1024.0 - 58.7               # -58.7: mean log-err correction

_CACHED = {}


def _build_nc(loop_n=1):
    nc = bacc.Bacc(None, target_bir_lowering=False)

    xT = nc.dram_tensor("xT", [D, S], F16, kind="ExternalInput")
    wqT = nc.dram_tensor("wqT", [D, DL], F16, kind="ExternalInput")
    wkT = nc.dram_tensor("wkT", [D, DL], F16, kind="ExternalInput")
    wvT = nc.dram_tensor("wvT", [D, DL], F16, kind="ExternalInput")
    woT = nc.dram_tensor("woT", [DL, D], F16, kind="ExternalInput")
    bq = nc.dram_tensor("bq", [DL], F32, kind="ExternalInput")
    bo = nc.dram_tensor("bo", [1, D], F16, kind="ExternalInput")
    y = nc.dram_tensor("y", [S, D], F16, kind="ExternalOutput")

    SHUF_MASK = [i % 4 for i in range(32)]

    with tile.TileContext(nc) as tc:
      for _rep in range(loop_n):
        with (
            tc.tile_pool(name="main", bufs=1) as pm,
            tc.tile_pool(name="qkt", bufs=2) as pqk,
            tc.tile_pool(name="wpair", bufs=2) as pw,
            tc.tile_pool(name="ptile", bufs=4) as ppt,
            tc.tile_pool(name="rtile", bufs=2) as prt,
            tc.tile_pool(name="ytile", bufs=3) as pyt,
            tc.tile_pool(name="psP", bufs=1, space="PSUM") as psP,
            tc.tile_pool(name="psS", bufs=2, space="PSUM") as psS,
            tc.tile_pool(name="psO", bufs=3, space="PSUM") as psO,
        ):
            # ---- persistent tiles
            xt = pm.tile([128, EC, S], F16, tag="xt")
            vt = pm.tile([128, ST, NH, VW], F16, tag="vt")
            ota = pm.tile([64, NPAIR, S], F16, tag="ota")   # even heads
            otb = pm.tile([64, NPAIR, S], F16, tag="otb")   # odd heads
            woa = pm.tile([64, NPAIR, D], F16, tag="woa")
            wob = pm.tile([64, NPAIR, D], F16, tag="wob")
            wvt = pm.tile([128, EC, DL], F16, tag="wvt")
            bqt = pm.tile([128, NPAIR], F32, tag="bqt")
            bot = pm.tile([1, D], F16, tag="bot")
            ones1h = pm.tile([1, 128], F16, tag="ones1h")

            nc.vector.memset(ones1h[:], 1.0)
            nc.vector.memset(vt[:, :, :, 64:VW], 1.0)
            nc.sync.dma_start(bqt[:], bq.ap().rearrange("(p d) -> d p", d=128))
            nc.sync.dma_start(bot[:], bo.ap())
            # x in (ec, s-half) pieces, s-half 0 first so phase_v starts early
            for sh in range(2):
                for ec in range(EC):
                    nc.sync.dma_start(
                        xt[:, ec, sh * 1024:(sh + 1) * 1024],
                        xT.ap()[ec * 128:(ec + 1) * 128,
                                sh * 1024:(sh + 1) * 1024])
            for ec in range(EC):
                nc.sync.dma_start(wvt[:, ec], wvT.ap()[ec * 128:(ec + 1) * 128, :])
            for c in range(NPAIR):
                nc.sync.dma_start(
                    woa[:, c], woT.ap()[(2 * c) * 64:(2 * c) * 64 + 64, :])
                nc.sync.dma_start(
                    wob[:, c], woT.ap()[(2 * c + 1) * 64:(2 * c + 1) * 64 + 64, :])

            # ---- phase V: V = x @ WvT, fp16, [s, d] layout + ones columns
            for st in range(ST):
                vps = psP.tile([128, 512], F32, tag="pj", name=f"vps{st}")
                for ec in range(EC):
                    nc.tensor.matmul(
                        vps[:], xt[:, ec, st * 128:(st + 1) * 128], wvt[:, ec],
                        start=(ec == 0), stop=(ec == EC - 1))
                nc.vector.tensor_copy(
                    vt[:, st, :, 0:64],
                    vps[:].rearrange("p (h c) -> p h c", h=NH))

            # ---- phase A: QT/KT per pair, fp16 [128 feats, S]
            def phase_a_gen(p, qt, kt, wqp, wkp):
                for wp, dst, bias in ((wqp, qt, bqt), (wkp, kt, None)):
                    for qc in range(4):
                        ps = psP.tile([128, 512], F32, tag="pj",
                                      name=f"aps{p}_{id(wp) % 97}_{qc}")
                        for ec in range(EC):
                            nc.tensor.matmul(
                                ps[:], wp[:, ec],
                                xt[:, ec, qc * 512:(qc + 1) * 512],
                                start=(ec == 0), stop=(ec == EC - 1))
                            yield
                        cs = slice(qc * 512, (qc + 1) * 512)
                        if bias is not None:
                            nc.vector.tensor_scalar_add(
                                dst[:, cs], ps[:], bias[:, p:p + 1])
                        else:
                            nc.vector.tensor_copy(dst[:, cs], ps[:])

            def start_pair(p):
                wqp = pw.tile([128, EC, 128], F16, tag="wqp", name=f"wqp{p}")
                wkp = pw.tile([128, EC, 128], F16, tag="wkp", name=f"wkp{p}")
                for ec in range(EC):
                    nc.sync.dma_start(
                        wqp[:, ec],
                        wqT.ap()[ec * 128:(ec + 1) * 128, p * 128:(p + 1) * 128])
                    nc.sync.dma_start(
                        wkp[:, ec],
                        wkT.ap()[ec * 128:(ec + 1) * 128, p * 128:(p + 1) * 128])
                qt = pqk.tile([128, S], F16, tag="qt", name=f"qt{p}")
                kt = pqk.tile([128, S], F16, tag="kt", name=f"kt{p}")
                return phase_a_gen(p, qt, kt, wqp, wkp), qt, kt

            # filler machinery: generators of PE work drained into
            # phase-B kti slots so the PE never idles while ACT runs exp
            class Fillers:
                def __init__(self):
                    self.gens = []

                def add(self, g):
                    self.gens.append(g)

                def step(self, n=1):
                    done = 0
                    while self.gens and done < n:
                        try:
                            next(self.gens[0])
                            done += 1
                        except StopIteration:
                            self.gens.pop(0)

                def drain(self):
                    while self.gens:
                        self.step(1000000)

            fill = Fillers()

            # pair 0 runs fully eager: unit (0, qb0) reads ALL of kt0 (every
            # key tile), so every k-projection write must be issued first —
            # a later issue would mean no RAW edge (read-before-write race).
            gen0, qt0, kt0 = start_pair(0)
            for _ in gen0:
                pass
            pair_qk = {0: (qt0, kt0)}

            SDEPTH = 2

            def phase_b_unit(p, qb, qt, kt):
                """One (pair, 512-query-block) unit: both heads row-packed."""
                he, ho = 2 * p, 2 * p + 1
                qa = qb * 512
                otpe = psO.tile([128, 512], F32, tag="otp",
                                name=f"otpe_{p}_{qb}")
                otpo = psO.tile([128, 512], F32, tag="otp",
                                name=f"otpo_{p}_{qb}")
                pts = []
                for step in range(ST + SDEPTH):
                    if step < ST:
                        kti = step
                        ks = slice(kti * 128, (kti + 1) * 128)
                        sc = psS.tile([128, 1024], F32, tag="sc",
                                      name=f"sc{p}_{qb}_{kti}")
                        nc.tensor.matmul(
                            sc[:, 0:512], kt[0:64, ks],
                            qt[0:64, qa:qa + 512], start=True, stop=True)
                        nc.tensor.matmul(
                            sc[:, 512:1024], kt[64:128, ks],
                            qt[64:128, qa:qa + 512], start=True, stop=True)
                        pt = ppt.tile([128, 1024], F16, tag="pt",
                                      name=f"pt{p}_{qb}_{kti}")
                        # split exp by head-half: even head exact on ACT,
                        # odd head Schraudolph on DVE: exp(s/8) ~=
                        # bitcast_f16(int16(s/8 * 1024/ln2 + B16)).  The odd
                        # head is schr for ALL keys, so its softmax ratio
                        # cancels the approximation's constant bias exactly.
                        nc.scalar.activation(
                            pt[:, 0:512], sc[:, 0:512],
                            mybir.ActivationFunctionType.Exp, scale=0.125)
                        nc.vector.tensor_scalar(
                            pt[:, 512:1024].bitcast(mybir.dt.int16),
                            sc[:, 512:1024], SCHR_A8, SCHR_B16,
                            op0=mybir.AluOpType.mult,
                            op1=mybir.AluOpType.add)
                        pts.append(pt)
                    if step >= SDEPTH:
                        t = step - SDEPTH
                        first, last = t == 0, t == ST - 1
                        nc.tensor.matmul(
                            otpe[0:VW, :], vt[:, t, he], pts[t][:, 0:512],
                            start=first, stop=last)
                        nc.tensor.matmul(
                            otpo[0:VW, :], vt[:, t, ho], pts[t][:, 512:1024],
                            start=first, stop=last)
                    fill.step(2)
                for otp, ot_dst in ((otpe, ota), (otpo, otb)):
                    r4 = prt.tile([128, 512], F16, tag="r4",
                                  name=f"r4_{p}_{qb}_{id(otp) % 97}")
                    rb = prt.tile([128, 512], F16, tag="rb",
                                  name=f"rb_{p}_{qb}_{id(otp) % 97}")
                    with nc.allow_low_precision(
                            reason="1/den in fp16; den<=6e3, rel err 5e-4"):
                        nc.vector.reciprocal(r4[64:68, :], otp[64:68, :])
                    nc.vector.stream_shuffle(
                        rb[0:32, :], r4[64:96, :], SHUF_MASK)
                    nc.vector.stream_shuffle(
                        rb[32:64, :], r4[64:96, :], SHUF_MASK)
                    nc.vector.tensor_tensor(
                        ot_dst[:, p, qa:qa + 512], otp[0:64, :],
                        rb[0:64, :], mybir.AluOpType.mult)

            def phase_c_gen(st_lo, st_hi):
                for st in range(st_lo, st_hi):
                    ss = slice(st * 128, (st + 1) * 128)
                    for eb in range(2):
                        es = slice(eb * 512, (eb + 1) * 512)
                        yps = psP.tile([128, 512], F32, tag="pj",
                                       name=f"yps{st}_{eb}")
                        nc.tensor.matmul(yps[:], ones1h[:], bot[:, es],
                                         start=True, stop=False)
                        yield
                        for c in range(NPAIR):
                            nc.tensor.matmul(
                                yps[:], ota[:, c, ss], woa[:, c, es],
                                start=False, stop=False)
                            yield
                            nc.tensor.matmul(
                                yps[:], otb[:, c, ss], wob[:, c, es],
                                start=False, stop=(c == NPAIR - 1))
                            yield
                        yt = pyt.tile([128, 512], F16, tag="yt",
                                      name=f"yt{st}_{eb}")
                        nc.vector.tensor_copy(yt[:], yps[:])
                        nc.sync.dma_start(y.ap()[ss, es], yt[:])

            for p in range(NPAIR):
                if p + 1 < NPAIR:
                    gnext, qtn, ktn = start_pair(p + 1)
                    pair_qk[p + 1] = (qtn, ktn)
                    fill.add(gnext)
                qt, kt = pair_qk[p]
                for qb in range(4):
                    if p == NPAIR - 1 and qb >= 1:
                        # st-blocks 4*(qb-1)..4*qb-1 complete once all pairs
                        # covered query block qb-1; absorb their out-proj
                        fill.add(phase_c_gen(4 * (qb - 1), 4 * qb))
                    phase_b_unit(p, qb, qt, kt)
                fill.drain()

            # ---- phase C tail: remaining s-tiles
            for _ in phase_c_gen(12, ST):
                pass

    nc.compile()
    return nc


def _get_nc(loop_n=1):
    key = f"nc{loop_n}"
    if key not in _CACHED:
        _CACHED[key] = _build_nc(loop_n)
    return _CACHED[key]


def make_in_maps(inputs):
    """Per-core input dict list from the full-problem input dict."""
    enc = np.asarray(inputs["encoder_input"], np.float32)
    in_maps = []
    for core in range(8):
        b, g = divmod(core, G)
        gs = slice(g * DL, (g + 1) * DL)
        Wo_g = np.asarray(inputs["Wo_w"], np.float64)[:, gs]
        bo_eff = np.asarray(inputs["Wv_b"], np.float64)[gs] @ Wo_g.T
        if g == 0:
            bo_eff = bo_eff + np.asarray(inputs["Wo_b"], np.float64)
        in_maps.append({
            "xT": np.ascontiguousarray(enc[b].T).astype(np.float16),
            "wqT": np.ascontiguousarray(
                np.asarray(inputs["Wq_w"], np.float32)[gs, :].T).astype(np.float16),
            "wkT": np.ascontiguousarray(
                np.asarray(inputs["Wk_w"], np.float32)[gs, :].T).astype(np.float16),
            "wvT": np.ascontiguousarray(
                np.asarray(inputs["Wv_w"], np.float32)[gs, :].T).astype(np.float16),
            "woT": np.ascontiguousarray(Wo_g.T).astype(np.float16),
            "bq": np.ascontiguousarray(
                np.asarray(inputs["Wq_b"], np.float32)[gs]),
            "bo": bo_eff.astype(np.float16).reshape(1, D),
        })
    return in_maps


def _get_runner():
    """Build the 8-core SPMD executable once and cache it."""
    if "runner" in _CACHED:
        return _CACHED["runner"]

    import jax
    from jax.sharding import Mesh, NamedSharding, PartitionSpec
    from jax.experimental.shard_map import shard_map
    from concourse import bass2jax
    from concourse.bass2jax import _bass_exec_p, install_neuronx_cc_hook

    nc = _get_nc()
    install_neuronx_cc_hook()
    partition_name = nc.partition_id_tensor.name if nc.partition_id_tensor else None
    in_names, out_names, out_avals, zero_outs = [], [], [], []
    for alloc in nc.m.functions[0].allocations:
        if not isinstance(alloc, mybir.MemoryLocationSet):
            continue
        name = alloc.memorylocations[0].name
        if alloc.kind == "ExternalInput":
            if name != partition_name:
                in_names.append(name)
        elif alloc.kind == "ExternalOutput":
            out_names.append(name)
            shape = tuple(alloc.tensor_shape)
            dtype = mybir.dt.np(alloc.dtype)
            out_avals.append(jax.core.ShapedArray(shape, dtype))
            zero_outs.append(np.zeros(shape, dtype))
    n_params, n_outs = len(in_names), len(out_avals)
    all_names = in_names + out_names + ([partition_name] if partition_name else [])

    def _body(*args):
        operands = list(args)
        if partition_name is not None:
            operands.append(bass2jax.partition_id_tensor())
        outs = _bass_exec_p.bind(
            *operands,
            out_avals=tuple(out_avals),
            in_names=tuple(all_names),
            out_names=tuple(out_names),
            lowering_input_output_aliases=(),
            sim_require_finite=True,
            sim_require_nnan=True,
            nc=nc,
        )
        return tuple(outs)

    devices = jax.devices()[:8]
    mesh = Mesh(np.asarray(devices), ("core",))
    f = jax.jit(
        shard_map(
            _body, mesh=mesh,
            in_specs=(PartitionSpec("core"),) * (n_params + n_outs),
            out_specs=(PartitionSpec("core"),) * n_outs,
            check_rep=False,
        ),
        donate_argnums=tuple(range(n_params, n_params + n_outs)),
        keep_unused=True,
    )
    shard = NamedSharding(mesh, PartitionSpec("core"))
    state = {
        "f": f, "in_names": in_names, "out_names": out_names,
        "zero_outs": zero_outs, "shard": shard, "jax": jax, "last_outs": None,
    }
    _CACHED["runner"] = state
    return state


def kernel(encoder_input, attention_mask, Wq_w, Wq_b, Wk_w, Wk_b, Wv_w, Wv_b,
           Wo_w, Wo_b):
    del attention_mask  # dead input in the reference forward
    inputs = {
        "encoder_input": encoder_input, "Wq_w": Wq_w, "Wq_b": Wq_b,
        "Wk_w": Wk_w, "Wk_b": Wk_b, "Wv_w": Wv_w, "Wv_b": Wv_b,
        "Wo_w": Wo_w, "Wo_b": Wo_b,
    }
    in_maps = make_in_maps(inputs)

    r = _get_runner()
    jax = r["jax"]

    concat_in = [
        jax.device_put(
            np.concatenate([in_maps[c][n] for c in range(8)], axis=0), r["shard"])
        for n in r["in_names"]
    ]
    outs = r["last_outs"]
    if outs is None:
        outs = [
            jax.device_put(
                np.zeros((8 * z.shape[0], *z.shape[1:]), z.dtype), r["shard"])
            for z in r["zero_outs"]
        ]
    outs = r["f"](*concat_in, *outs)
    np_outs = [np.asarray(o) for o in outs]
    r["last_outs"] = list(outs)

    per_core = {}
    for i, nme in enumerate(r["out_names"]):
        full = np_outs[i].reshape(8, -1, *np_outs[i].shape[1:])
        per_core[nme] = full

    yv = per_core["y"]
    out = np.empty((B, S, D), dtype=np.float32)
    for b in range(B):
        out[b] = yv[G * b].astype(np.float32) + yv[G * b + 1].astype(np.float32)
    return out
